# revision 42
# baseline (speedup 1.0000x reference)
"""GQA attention (B=2, T=2048, DIM=2048, NH=32, NKV=8, HD=64) with RoPE, causal,
on 8 TRN2 NeuronCores.

Sharding: data-parallel over B (2) x tensor-parallel over kv-head groups (4).
Core i handles batch i//4 and kv heads {2g, 2g+1} (g = i%4), i.e. q heads
8g..8g+8. wq/wk/wv column-parallel, wo row-parallel; host sums the 4 partial
outputs per batch.

Everything on-device is feature-major ("transposed"): x^T, Q^T, K^T are
[feature, t] so no on-device transposes are needed anywhere:
  QT[d,t] = wq^T x^T;  scoresT[s,q] = (KT slice)^T @ QT;  OT[d,q] = V^T @ PT;
  outT[o,t] = wo^T @ OT.  Host transposes the final [o,t] back to [t,o].

RoPE de-interleave: within each head the 64 features are permuted to
[32 real | 32 imag] (host permutes wq/wk columns), so rot = q*C + swap(q)*S
with the half-swap done by SBUF->SBUF DMA.

Head->row layout: slab s (of 4) holds q heads (8g+s) at rows 0:64 and
(8g+s+4) at rows 64:128, so the two heads of a slab use kv0/kv1 = rows
0:64/64:128 of KT directly.  The two heads' 64-contraction score matmuls are
emitted interleaved so they land adjacently in the PE queue and co-execute in
PE row-groups (0,0)/(64,0) - halving score cost vs serial emission.

Softmax: no max subtraction (|score| <~ 6 after the 1/8 scale folded into wq).
The PV stationary operand is [V (64 cols) | ones (64 cols)] per 128-row
s-block, so rows 64:128 of the PV accumulator hold 64 broadcast copies of the
softmax denominator - the reciprocal+normalize then run as plain 64-partition
DVE ops (no partition broadcast needed).

Causal mask: DVE multiply by 0/1 tiles on diagonal score pairs only;
fully-masked tiles are skipped, and the second diagonal pair of each q-chunk
only computes q in [256,512) (the lower half is entirely masked).  Scores are
computed in side-by-side pairs in a 2-bank PSUM tile so one ACT exp
instruction covers up to 1024 columns.

Emission order per t-block: attention, then wo, then next block's
projections - the projection/wo matmuls get later scheduler priority so they
fill PE bubbles in the ACT-bound exp stream without splitting the score-pair
row-group packing.

Input DMAs are batched multi-chunk descriptors spread over the sync/scalar/
gpsimd queues, staged so K/V/Q weights and the first x block arrive first.

All matmul operands are fp16 (KERNEL_MM_DTYPE also allows bf16/f32r); the PE
streams 2-byte operands at full rate, fp32 PSUM accumulate.
"""

import numpy as np

B, T, DIM = 2, 2048, 2048
NH, NKV, HD = 32, 8, 64
G = 4            # tensor-parallel groups
QH = NH // G     # 8 local q heads
SLABS = 4
KTILES = DIM // 128
TBLK = T // 512

_CACHE = {}
import os as _os
_MM_DTYPE = _os.environ.get("KERNEL_MM_DTYPE", "fp16")


def _to_mm_dtype(x: np.ndarray) -> np.ndarray:
    if _MM_DTYPE == "bf16":
        import ml_dtypes
        return np.ascontiguousarray(x, dtype=np.float32).astype(ml_dtypes.bfloat16)
    if _MM_DTYPE == "fp16":
        return np.ascontiguousarray(x, dtype=np.float32).astype(np.float16)
    return _round_f32r(x)


def _round_f32r(x: np.ndarray) -> np.ndarray:
    """Round f32 to the float32r grid (11 mantissa bits, round-to-nearest-even)."""
    x = np.ascontiguousarray(x, dtype=np.float32)
    xi = x.view(np.uint32).copy()
    shift = 12  # keep 11 mantissa bits
    lsb = (xi >> shift) & 1
    xi = (xi + ((1 << (shift - 1)) - 1) + lsb) & np.uint32(~((1 << shift) - 1) & 0xFFFFFFFF)
    return xi.view(np.float32)


def _build():
    import concourse.bass as bass
    import concourse.mybir as mybir
    import concourse.tile as tile
    from concourse import bacc

    F32 = mybir.dt.float32
    F32R = {"bf16": mybir.dt.bfloat16, "fp16": mybir.dt.float16,
            "f32r": mybir.dt.float32r}[_MM_DTYPE]
    F8 = mybir.dt.float8e4
    DR = mybir.MatmulPerfMode.DoubleRow
    EXP = mybir.ActivationFunctionType.Exp

    nc = bacc.Bacc("TRN2", target_bir_lowering=False, debug=False, num_devices=8)

    xT = nc.dram_tensor("xT", [DIM, T], F32R, kind="ExternalInput").ap()
    wq = nc.dram_tensor("wq", [DIM, QH * HD], F32R, kind="ExternalInput").ap()
    wk = nc.dram_tensor("wk", [DIM, 2 * HD], F32R, kind="ExternalInput").ap()
    wv = nc.dram_tensor("wv", [DIM, 2 * HD], F32R, kind="ExternalInput").ap()
    wo = nc.dram_tensor("wo", [QH * HD, DIM], F32R, kind="ExternalInput").ap()
    c4 = nc.dram_tensor("c4", [128, T], F32R, kind="ExternalInput").ap()
    s4 = nc.dram_tensor("s4", [128, T], F32R, kind="ExternalInput").ap()
    vones = nc.dram_tensor("vones", [128, 16 * 128], F32R, kind="ExternalInput").ap()
    msk0 = nc.dram_tensor("msk0", [128, 1024], F32R, kind="ExternalInput").ap()
    msk1 = nc.dram_tensor("msk1", [128, 512], F32R, kind="ExternalInput").ap()
    outT = nc.dram_tensor("outT", [DIM, T], F32R, kind="ExternalOutput").ap()

    def dma_blk(eng, dst2d, src2d):
        # DMA [N*128, F] DRAM <-> [128, N*F] SBUF (row-block n at cols n*F)
        n = src2d.shape[0] // 128
        eng.dma_start(dst2d.rearrange("p (n f) -> p n f", n=n),
                      src2d.rearrange("(n p) f -> p n f", p=128))

    def dma_blk_out(eng, dst2d, src2d):
        n = dst2d.shape[0] // 128
        eng.dma_start(dst2d.rearrange("(n p) f -> p n f", p=128),
                      src2d.rearrange("p (n f) -> p n f", n=n))

    from contextlib import ExitStack

    with tile.TileContext(nc) as tc, ExitStack() as ctx:
        # ---------- persistent tiles ----------
        pers = ctx.enter_context(tc.tile_pool(name="pers", bufs=1))
        KT = pers.tile([128, T], F32R, tag="kt", name="kt")
        VO0 = pers.tile([128, 16 * 128], F32R, tag="vo0", name="vo0")
        VO1 = pers.tile([128, 16 * 128], F32R, tag="vo1", name="vo1")
        MSK0 = pers.tile([128, 1024], F32R, tag="msk0", name="msk0_sb")
        MSK1 = pers.tile([128, 512], F32R, tag="msk1", name="msk1_sb")
        C4 = pers.tile([128, T], F32R, tag="c4", name="c4_sb")
        S4 = pers.tile([128, T], F32R, tag="s4", name="s4_sb")
        WQ = pers.tile([128, SLABS * 2048], F32R, tag="wq", name="wq_sb")
        WK = pers.tile([128, KTILES * 128], F32R, tag="wk", name="wk_sb")
        WV = pers.tile([128, KTILES * 128], F32R, tag="wv", name="wv_sb")
        WO = [pers.tile([128, T], F32R, tag=f"wo{s}", name=f"wo{s}") for s in range(SLABS)]

        rot = ctx.enter_context(tc.tile_pool(name="rot", bufs=2))
        work = ctx.enter_context(tc.tile_pool(name="work", bufs=3))
        ptp = ctx.enter_context(tc.tile_pool(name="ptp", bufs=6))
        rcp = ctx.enter_context(tc.tile_pool(name="rcp", bufs=2))
        osbp = ctx.enter_context(tc.tile_pool(name="osbp", bufs=2))
        xtp = ctx.enter_context(tc.tile_pool(name="xt", bufs=2))
        ps_acc = ctx.enter_context(tc.tile_pool(name="ps_acc", bufs=2, space="PSUM"))
        ps_sc = ctx.enter_context(tc.tile_pool(name="ps_sc", bufs=2, space="PSUM"))
        ps_ot = ctx.enter_context(tc.tile_pool(name="ps_ot", bufs=1, space="PSUM"))

        # ---------- startup DMAs, staged by first use ----------
        # Startup staging: each issue queue drains its DMAs in order and
        # transfers contend for shared DMA engines, so only critical-path
        # bytes go first and the scalar queue carries NO DMAs at all (a
        # dma_start occupies the issuing queue and would stall the exp
        # stream behind it).
        xts0 = xtp.tile([128, KTILES * 512], F32R, tag="x", name="xt0")
        for h in range(2):
            dma_blk(nc.sync, WK[:, h * 1024:(h + 1) * 1024],
                    wk[h * 1024:(h + 1) * 1024, :])
        for c in range(4):
            dma_blk(nc.sync, xts0[:, c * 2048:(c + 1) * 2048],
                    xT[c * 512:(c + 1) * 512, 0:512])
        nc.sync.dma_start(MSK0[:], msk0[:])
        nc.sync.dma_start(MSK1[:], msk1[:])
        nc.sync.dma_start(VO0[:], vones[:])
        nc.sync.dma_start(VO1[:], vones[:])
        nc.gpsimd.dma_start(C4[:, 0:512], c4[:, 0:512])
        nc.gpsimd.dma_start(S4[:, 0:512], s4[:, 0:512])
        for h in range(2):
            dma_blk(nc.gpsimd, WQ[:, h * 1024:(h + 1) * 1024],
                    wq[h * 1024:(h + 1) * 1024, 0:128])
        for s in range(1, SLABS):
            for h in range(2):
                dma_blk(nc.gpsimd,
                        WQ[:, s * 2048 + h * 1024: s * 2048 + (h + 1) * 1024],
                        wq[h * 1024:(h + 1) * 1024, s * 128:(s + 1) * 128])
        for h in range(2):
            dma_blk(nc.gpsimd, WV[:, h * 1024:(h + 1) * 1024],
                    wv[h * 1024:(h + 1) * 1024, :])

        # warm up the ACT exp table during the DMA phase (first ACTIVATE
        # triggers a ~2.7us table load; tie it to an early small DMA)
        warm = rcp.tile([1, 16], F32, tag="warm", name="warm")
        nc.scalar.activation(warm[:], C4[0:1, 0:16], EXP)

        def rope(ps, dst, t_sl):
            q_sb = work.tile([128, 512], F32, tag="qsb", name="qsb")
            nc.vector.tensor_copy(q_sb[:], ps[:])
            q_sw = work.tile([128, 512], F32, tag="qsw", name="qsw")
            for o in (0, 64):
                nc.gpsimd.dma_start(q_sw[o:o + 32, :], q_sb[o + 32:o + 64, :])
                nc.gpsimd.dma_start(q_sw[o + 32:o + 64, :], q_sb[o:o + 32, :])
            m1 = work.tile([128, 512], F32, tag="m1", name="m1")
            nc.vector.tensor_mul(m1[:], ps[:], C4[:, t_sl])
            m2 = work.tile([128, 512], F32, tag="m2", name="m2")
            nc.vector.tensor_mul(m2[:], q_sw[:], S4[:, t_sl])
            nc.vector.tensor_add(dst, m1[:], m2[:])

        def emit_proj(tb):
            t_sl = slice(tb * 512, (tb + 1) * 512)
            if tb == 0:
                xts = xts0
            else:
                xts = xtp.tile([128, KTILES * 512], F32R, tag="x", name=f"xt{tb}")
                dma_blk(nc.sync, xts[:], xT[:, t_sl])
            # K projection first: KT gates the next attention block
            ps = ps_acc.tile([128, 512], F32, tag="acc", name="pk")
            for k in range(KTILES):
                nc.tensor.matmul(ps[:], WK[:, k * 128:(k + 1) * 128],
                                 xts[:, k * 512:(k + 1) * 512],
                                 start=(k == 0), stop=(k == KTILES - 1))
            rope(ps[:], KT[:, t_sl], t_sl)
            # Q projections
            QTr = []
            for s in range(SLABS):
                ps = ps_acc.tile([128, 512], F32, tag="acc", name="pq")
                for k in range(KTILES):
                    nc.tensor.matmul(ps[:], WQ[:, s * 2048 + k * 128: s * 2048 + (k + 1) * 128],
                                     xts[:, k * 512:(k + 1) * 512],
                                     start=(k == 0), stop=(k == KTILES - 1))
                dst_t = rot.tile([128, 512], F32R, tag=f"qtr{s}", name=f"qtr{s}")
                QTr.append(dst_t)
                rope(ps[:], dst_t[:], t_sl)
            # V projection last: out [t,128] blocks into the [V|ones] tiles
            for i in range(4):
                sbi = tb * 4 + i
                pv = ps_acc.tile([128, 128], F32, tag="acc", name="pv",
                                 padded_shape=[128, 512])
                for k in range(KTILES):
                    nc.tensor.matmul(pv[:],
                                     xts[:, k * 512 + i * 128: k * 512 + (i + 1) * 128],
                                     WV[:, k * 128:(k + 1) * 128],
                                     start=(k == 0), stop=(k == KTILES - 1))
                nc.vector.tensor_copy(VO0[:, sbi * 128: sbi * 128 + 64], pv[:, 0:64])
                nc.vector.tensor_copy(VO1[:, sbi * 128: sbi * 128 + 64], pv[:, 64:128])
            return QTr

        def emit_attention(qc, QTr):
            npair = 2 * qc + 2
            OTN = []
            for s in range(SLABS):
                ot0 = ps_ot.tile([128, 512], F32, tag="ot0", name="ot0")
                ot1 = ps_ot.tile([128, 512], F32, tag="ot1", name="ot1")
                for pr in range(npair):
                    diag0 = (pr == 2 * qc)       # full-width diagonal pair
                    diag1 = (pr == 2 * qc + 1)   # narrow diagonal pair
                    W = 256 if diag1 else 512
                    qoff = 256 if diag1 else 0
                    sb0, sb1 = 2 * pr, 2 * pr + 1
                    sc0 = ps_sc.tile([128, 1024], F32, tag="sc", name="sc0")
                    sc1 = ps_sc.tile([128, 1024], F32, tag="sc", name="sc1")
                    # interleave the two 64-row halves so they pack into
                    # PE row-groups (0,0)/(64,0) and co-execute
                    nc.tensor.matmul(sc0[:, 0:W],
                                     KT[0:64, sb0 * 128:(sb0 + 1) * 128],
                                     QTr[s][0:64, qoff:512], start=True, stop=True)
                    nc.tensor.matmul(sc1[:, 0:W],
                                     KT[64:128, sb0 * 128:(sb0 + 1) * 128],
                                     QTr[s][64:128, qoff:512], start=True, stop=True)
                    nc.tensor.matmul(sc0[:, W:2 * W],
                                     KT[0:64, sb1 * 128:(sb1 + 1) * 128],
                                     QTr[s][0:64, qoff:512], start=True, stop=True)
                    nc.tensor.matmul(sc1[:, W:2 * W],
                                     KT[64:128, sb1 * 128:(sb1 + 1) * 128],
                                     QTr[s][64:128, qoff:512], start=True, stop=True)
                    pt0 = ptp.tile([128, 1024], F32R, tag="pt", name="pt0")
                    pt1 = ptp.tile([128, 1024], F32R, tag="pt", name="pt1")
                    nc.scalar.activation(pt0[:, 0:2 * W], sc0[:, 0:2 * W], EXP)
                    nc.scalar.activation(pt1[:, 0:2 * W], sc1[:, 0:2 * W], EXP)
                    if diag0:
                        nc.vector.tensor_mul(pt0[:, 0:1024], pt0[:, 0:1024], MSK0[:])
                        nc.vector.tensor_mul(pt1[:, 0:1024], pt1[:, 0:1024], MSK0[:])
                    elif diag1:
                        nc.vector.tensor_mul(pt0[:, 0:512], pt0[:, 0:512], MSK1[:])
                        nc.vector.tensor_mul(pt1[:, 0:512], pt1[:, 0:512], MSK1[:])
                    st, sp = (pr == 0), (pr == npair - 1)
                    nc.tensor.matmul(ot0[:, qoff:512], VO0[:, sb0 * 128:(sb0 + 1) * 128],
                                     pt0[:, 0:W], start=st, stop=False)
                    nc.tensor.matmul(ot1[:, qoff:512], VO1[:, sb0 * 128:(sb0 + 1) * 128],
                                     pt1[:, 0:W], start=st, stop=False)
                    nc.tensor.matmul(ot0[:, qoff:512], VO0[:, sb1 * 128:(sb1 + 1) * 128],
                                     pt0[:, W:2 * W], start=False, stop=sp)
                    nc.tensor.matmul(ot1[:, qoff:512], VO1[:, sb1 * 128:(sb1 + 1) * 128],
                                     pt1[:, W:2 * W], start=False, stop=sp)
                # normalize: rows 64:128 of ot hold 64 copies of the denominator.
                # (reciprocal_approx_fast is a custom DVE op and only works at
                # partition base 0 -> stage denominators into a base-0 tile.)
                otn = rot.tile([128, 512], F32R, tag=f"otn{s}", name=f"otn{s}")
                OTN.append(otn)
                dcp = rcp.tile([128, 512], F32, tag="dcp", name="dcp")
                nc.vector.tensor_copy(dcp[0:64, :], ot0[64:128, :])
                nc.vector.tensor_copy(dcp[64:128, :], ot1[64:128, :])
                rec = rcp.tile([128, 512], F32, tag="rec", name="rec")
                nc.vector.reciprocal_approx_fast(rec[:], dcp[:])
                nc.vector.tensor_mul(otn[0:64, :], ot0[0:64, :], rec[0:64, :])
                nc.vector.tensor_mul(otn[64:128, :], ot1[0:64, :], rec[64:128, :])
            return OTN

        def emit_wo(tb, OTN):
            t_sl = slice(tb * 512, (tb + 1) * 512)
            last = (tb == TBLK - 1)
            # at the tail (tb3) the scalar engine is free after the last exp:
            # finer store groups alternating the two HWDGE queues + ACT-side
            # PSUM evacuations shorten the output drain
            go = 2 if last else 4
            for g in range(16 // go):
                osb = osbp.tile([128, go * 512], F32R, tag="osb", name="osb")
                for i in range(go):
                    ob = g * go + i
                    po = ps_acc.tile([128, 512], F32, tag="acc", name="po")
                    for s in range(SLABS):
                        nc.tensor.matmul(po[:], WO[s][:, ob * 128:(ob + 1) * 128],
                                         OTN[s][:], start=(s == 0), stop=(s == SLABS - 1))
                    if last and ob % 2 == 1:
                        nc.scalar.copy(osb[:, i * 512:(i + 1) * 512], po[:])
                    else:
                        nc.vector.tensor_copy(osb[:, i * 512:(i + 1) * 512], po[:])
                eng = (nc.sync if g % 2 == 0 else nc.scalar) if last else nc.sync
                dma_blk_out(eng, outT[g * go * 128:(g + 1) * go * 128, t_sl], osb[:])

        QTr = emit_proj(0)
        # deferred non-critical loads: later priority than proj(0)'s rope
        # swaps so they don't crowd the gpsimd queue during startup
        nc.gpsimd.dma_start(C4[:, 512:T], c4[:, 512:T])
        nc.gpsimd.dma_start(S4[:, 512:T], s4[:, 512:T])
        for s in range(SLABS):
            nc.gpsimd.dma_start(WO[s][:], wo[s * 128:(s + 1) * 128, :])
        for tb in range(TBLK):
            QTr_next = None
            # attention gets strongly-early priority: when an attention
            # instruction and a filler instruction are both ready, attention
            # wins - keeping the score-pair row-group packing intact
            with tc.high_priority(offset=1 << 20):
                OTN = emit_attention(tb, QTr)
            # proj before wo: the shared acc-pool FIFO then lets next-block
            # projections start during this block's attention
            if tb + 1 < TBLK:
                QTr_next = emit_proj(tb + 1)
            emit_wo(tb, OTN)
            QTr = QTr_next

    nc.compile()
    return nc


def _prep_inputs(x, freqs_cos, freqs_sin, wq, wk, wv, wo):
    """Build the 8 per-core input maps (host-side sharding + layout prep)."""
    x = np.asarray(x, dtype=np.float32)
    freqs_cos = np.asarray(freqs_cos, dtype=np.float32)
    freqs_sin = np.asarray(freqs_sin, dtype=np.float32)
    wq = np.asarray(wq, dtype=np.float32)
    wk = np.asarray(wk, dtype=np.float32)
    wv = np.asarray(wv, dtype=np.float32)
    wo = np.asarray(wo, dtype=np.float32)

    # de-interleave permutation within a head: [2j] then [2j+1]
    deint = np.concatenate([np.arange(0, HD, 2), np.arange(1, HD, 2)])

    # rope tables [128, T]: row r uses freq index r % 32; sign of sin flips
    # per 32-block (real-out blocks get -sin)
    cosT = freqs_cos.T  # [32, T]
    sinT = freqs_sin.T
    c4 = np.tile(cosT, (4, 1)).astype(np.float32)
    s4 = np.concatenate([-sinT, sinT, -sinT, sinT], axis=0).astype(np.float32)

    # [V(64) | ones(64)] stationary-tile init: ones in cols 64:128 per block
    vones = np.zeros((128, 16 * 128), dtype=np.float32)
    vones.reshape(128, 16, 128)[:, :, 64:] = 1.0
    # diagonal masks:
    #  msk0 (full pair, s-blocks 4qc,4qc+1 vs q in [0,512)):
    #    msk0[p, j*512 + q] = (128j + p) <= q
    #  msk1 (narrow pair, s-blocks 4qc+2,4qc+3 vs q in [256,512)):
    #    msk1[p, j*256 + qq] = (128j + p) <= qq
    p_ = np.arange(128)[:, None]
    msk0 = np.zeros((128, 1024), dtype=np.float32)
    msk1 = np.zeros((128, 512), dtype=np.float32)
    for j in range(2):
        q_ = np.arange(512)[None, :]
        msk0[:, j * 512:(j + 1) * 512] = (128 * j + p_) <= q_
        qq_ = np.arange(256)[None, :]
        msk1[:, j * 256:(j + 1) * 256] = (128 * j + p_) <= qq_

    in_maps = []
    for core in range(8):
        b, g = divmod(core, 4)
        # local q head order: slab-major, (s, half) -> global head 8g + s + 4*half
        qheads = [8 * g + s + 4 * h for s in range(SLABS) for h in range(2)]
        kvheads = [2 * g, 2 * g + 1]

        wq_cols = np.concatenate([qh * HD + deint for qh in qheads])
        wk_cols = np.concatenate([kh * HD + deint for kh in kvheads])
        wv_cols = np.concatenate([np.arange(kh * HD, (kh + 1) * HD) for kh in kvheads])
        wo_rows = np.concatenate([np.arange(qh * HD, (qh + 1) * HD) for qh in qheads])

        in_maps.append({
            "xT": _to_mm_dtype(x[b].T),
            "wq": _to_mm_dtype(wq[:, wq_cols] * (1.0 / np.sqrt(HD))),
            "wk": _to_mm_dtype(wk[:, wk_cols]),
            "wv": _to_mm_dtype(wv[:, wv_cols]),
            "wo": _to_mm_dtype(wo[wo_rows, :]),
            "c4": _to_mm_dtype(c4),
            "s4": _to_mm_dtype(s4),
            "vones": _to_mm_dtype(vones),
            "msk0": _to_mm_dtype(msk0),
            "msk1": _to_mm_dtype(msk1),
        })
    return in_maps


def kernel(x, freqs_cos, freqs_sin, wq, wk, wv, wo, _trace=False):
    from concourse.bass_utils import run_bass_kernel_spmd

    if "nc" not in _CACHE:
        _CACHE["nc"] = _build()
    nc = _CACHE["nc"]

    in_maps = _prep_inputs(x, freqs_cos, freqs_sin, wq, wk, wv, wo)
    res = run_bass_kernel_spmd(nc, in_maps, core_ids=list(range(8)), trace=_trace)
    _CACHE["last_result"] = res

    out = np.empty((B, T, DIM), dtype=np.float32)
    for b in range(B):
        acc = res.results[4 * b]["outT"].astype(np.float32)
        for g in range(1, 4):
            acc = acc + res.results[4 * b + g]["outT"].astype(np.float32)
        out[b] = acc.T
    return out
